# revision 4
# baseline (speedup 1.0000x reference)
"""Trainium2 Bass kernel for nn_DecompMultiTransform (RGCN basis-decomposition).

Reference computation:
    full_w = (w_comp @ weight).reshape(64, 256, 256)   # per-type weights
    out[n, :] = x[n, :] @ full_w[xtype[n]]             # N = 4096

Kernel formulation (avoids materializing the 16 MB full_w and the 1 GB
per-sample weight gather):
    c[n, b]   = w_comp[xtype[n], b]                    # [N, 16] tiny gather
    u[(b,i),n] = c[n, b] * x[n, i]                     # scaled copies of x
    outT[o,n] = sum_{b,i} weight[b, i*256+o] * u[(b,i), n]
i.e. one dense K=4096 matmul per core after a cheap elementwise scale.

Sharding: data-parallel over N across 8 cores (512 rows each); weight,
w_comp replicated. x is uploaded transposed ([256, 512] per core) and the
output comes back transposed ([256, 512]) — pure layout choices done at
shard time on host. All math (gather, scale, matmuls) runs on device.

Matmuls run in float32r (TRN2's full-rate fp32 mode, ~1.2e-4 rounding).
"""

import sys

if "/opt/trn_rl_repo" not in sys.path:
    sys.path.insert(0, "/opt/trn_rl_repo")

import numpy as np

import concourse.bass as bass
import concourse.mybir as mybir
import concourse.tile as tile
from concourse import bacc
from concourse.bass_utils import run_bass_kernel_spmd
from concourse.masks import make_identity

P = 128
N_FULL = 4096
IN_DIM = 256
OUT_DIM = 256
NUM_B = 16
NUM_T = 64
N_CORES = 8
ROWS = N_FULL // N_CORES          # 512 rows per core
NT = ROWS // P                    # 4 row tiles
KT = NUM_B * (IN_DIM // P)        # 32 contraction tiles of 128
GPS_BASES = frozenset({2, 6, 10, 13})  # bases whose TT runs on gpsimd

F32 = mybir.dt.float32
F32R = mybir.dt.float32r
I32 = mybir.dt.int32


def _build_program():
    nc = bacc.Bacc("TRN2", target_bir_lowering=False, debug=False)

    xT = nc.declare_dram_parameter("xT", [P, 2 * ROWS], F32, isOutput=False)
    xtype = nc.declare_dram_parameter("xtype", [ROWS, 1], I32, isOutput=False)
    w_comp = nc.declare_dram_parameter("w_comp", [NUM_T, NUM_B], F32, isOutput=False)
    weight = nc.declare_dram_parameter("weight", [NUM_B, IN_DIM * OUT_DIM], F32R, isOutput=False)
    sel_in = nc.declare_dram_parameter("sel_in", [NUM_B, NUM_B * P], F32R, isOutput=False)
    outT = nc.declare_dram_parameter("outT", [OUT_DIM, ROWS], F32, isOutput=True)

    # weight viewed as [b][p=i%128][ih=i//128*? ...]: chunk per b is
    # [128, 2, 256] with w_chunk[b][p, ih, o] = weight[b, (ih*128+p)*256 + o]
    wv = weight.ap().rearrange("b (ih p o) -> b p ih o", ih=2, p=P, o=OUT_DIM)

    with tile.TileContext(nc) as tc:
        with (
            tc.tile_pool(name="const", bufs=1) as constp,
            tc.tile_pool(name="wpool", bufs=1) as wpool,
            tc.tile_pool(name="cbp", bufs=1) as cbp,
            tc.tile_pool(name="cp", bufs=4) as cp,
            tc.tile_pool(name="up", bufs=5) as up,
            tc.tile_pool(name="outp", bufs=2) as outp,
            tc.tile_pool(name="pst", bufs=2, space="PSUM") as pst,
            tc.tile_pool(name="psb", bufs=3, space="PSUM") as psb,
            tc.tile_pool(name="pso", bufs=1, space="PSUM") as pso,
        ):
            # ---- constants / inputs ----
            sel = constp.tile([NUM_B, NUM_B * P], F32R, name="sel")
            nc.sync.dma_start(out=sel[:], in_=sel_in.ap()[:, :])

            identity = constp.tile([P, P], F32, name="identity")
            make_identity(nc, identity[:])

            # tiny idx DMAs first so the gather pipeline starts immediately
            idxts = []
            for nt in range(NT):
                idxt = cp.tile([P, 1], I32, name="idxt", tag="idxt", bufs=4)
                nc.sync.dma_start(out=idxt, in_=xtype.ap()[nt * P : (nt + 1) * P, :])
                idxts.append(idxt)

            xtcat = constp.tile([P, 2 * ROWS], F32, name="xtcat")
            nc.sync.dma_start(out=xtcat, in_=xT.ap()[:, :])

            # weight chunks, resident; split across the two HWDGE queues
            wts = []
            for b in range(NUM_B):
                wt = wpool.tile([P, 2, OUT_DIM], F32R, name=f"w{b}")
                eng = nc.sync if b % 2 == 0 else nc.scalar
                eng.dma_start(out=wt, in_=wv[b])
                wts.append(wt)

            # ---- c gather + transpose to cT [16, 512] ----
            cT = constp.tile([NUM_B, ROWS], F32R, name="cT")
            for nt in range(NT):
                cnat = cp.tile([P, NUM_B], F32, name="cnat", tag="cnat")
                nc.gpsimd.indirect_dma_start(
                    out=cnat[:],
                    out_offset=None,
                    in_=w_comp.ap()[:, :],
                    in_offset=bass.IndirectOffsetOnAxis(ap=idxts[nt][:, :1], axis=0),
                )
                ctps = pst.tile([NUM_B, P], F32, name="ctps", tag="ctps", space="PSUM")
                nc.tensor.transpose(out=ctps[:], in_=cnat[:], identity=identity[:])
                nc.vector.tensor_copy(cT[:, nt * P : (nt + 1) * P], ctps[:])

            # ---- per-basis: broadcast c row, scale both x halves, accumulate ----
            psums = [
                pso.tile([P, ROWS], F32, name=f"out{oh}", space="PSUM")
                for oh in range(2)
            ]
            for b in range(NUM_B):
                cbp_ps = psb.tile([P, ROWS], F32, name="cbps", tag="cbps", space="PSUM")
                nc.tensor.matmul(
                    out=cbp_ps[:],
                    lhsT=sel[:, b * P : (b + 1) * P],
                    rhs=cT[:],
                    start=True,
                    stop=True,
                )
                use_gps = b in GPS_BASES
                if use_gps:
                    # gpsimd cannot read PSUM; stage via scalar engine
                    cb_sb = cbp.tile([P, ROWS], F32, name="cbsb", tag="cbsb", bufs=2)
                    nc.scalar.copy(cb_sb[:], cbp_ps[:])
                    cb_src = cb_sb
                else:
                    cb_src = cbp_ps
                cb_rep = cb_src[:].rearrange("p (one n) -> p one n", one=1).to_broadcast(
                    [P, 2, ROWS]
                )
                u = up.tile([P, 2 * ROWS], F32R, name="u", tag="u")
                eng = nc.gpsimd if use_gps else nc.vector
                eng.tensor_tensor(
                    out=u[:].rearrange("p (ih n) -> p ih n", ih=2),
                    in0=xtcat[:].rearrange("p (ih n) -> p ih n", ih=2),
                    in1=cb_rep,
                    op=mybir.AluOpType.mult,
                )
                for ih in range(2):
                    kt = b * 2 + ih
                    for oh in range(2):
                        nc.tensor.matmul(
                            out=psums[oh][:],
                            lhsT=wts[b][:, ih, oh * P : (oh + 1) * P],
                            rhs=u[:, ih * ROWS : (ih + 1) * ROWS],
                            start=(kt == 0),
                            stop=(kt == KT - 1),
                        )

            # ---- drain outT ----
            for oh in range(2):
                ot = outp.tile([P, ROWS], F32, name=f"ot{oh}")
                nc.scalar.copy(ot[:], psums[oh][:])
                eng = nc.sync if oh == 0 else nc.scalar
                eng.dma_start(out=outT.ap()[oh * P : (oh + 1) * P, :], in_=ot)

    nc.compile()
    return nc


_PROGRAM = None
LAST_RESULT = None  # test harness introspection


def _sel_np():
    sel = np.zeros((NUM_B, NUM_B * P), np.float32)
    for b in range(NUM_B):
        sel[b, b * P : (b + 1) * P] = 1.0
    return sel


def kernel(x, xtype, weight, w_comp, trace=False):
    global _PROGRAM, LAST_RESULT
    x = np.asarray(x, dtype=np.float32)
    xtype = np.asarray(xtype)
    weight = np.asarray(weight, dtype=np.float32)
    w_comp = np.asarray(w_comp, dtype=np.float32)
    assert x.shape == (N_FULL, IN_DIM) and weight.shape == (NUM_B, IN_DIM * OUT_DIM)

    if _PROGRAM is None:
        _PROGRAM = _build_program()
    nc = _PROGRAM

    sel = _sel_np()
    xtype32 = xtype.astype(np.int32).reshape(N_FULL, 1)
    in_maps = []
    for c in range(N_CORES):
        s = slice(c * ROWS, (c + 1) * ROWS)
        in_maps.append(
            {
                "xT": np.ascontiguousarray(
                    x[s].T.reshape(2, P, ROWS).transpose(1, 0, 2).reshape(P, 2 * ROWS)
                ),
                "xtype": np.ascontiguousarray(xtype32[s]),
                "w_comp": w_comp,
                "weight": weight,
                "sel_in": sel,
            }
        )

    res = run_bass_kernel_spmd(nc, in_maps, list(range(N_CORES)), trace=trace)
    LAST_RESULT = res

    out = np.empty((N_FULL, OUT_DIM), np.float32)
    for c in range(N_CORES):
        s = slice(c * ROWS, (c + 1) * ROWS)
        out[s] = res.results[c]["outT"].T
    return out


# revision 5
# speedup vs baseline: 1.2824x; 1.2824x over previous
"""Trainium2 Bass kernel for nn_DecompMultiTransform (RGCN basis-decomposition).

Reference computation:
    full_w = (w_comp @ weight).reshape(64, 256, 256)   # per-type weights
    out[n, :] = x[n, :] @ full_w[xtype[n]]             # N = 4096

Kernel formulation (avoids materializing the 16 MB full_w and the 1 GB
per-sample weight gather):
    onehot[t, n] = (xtype[n] == t)                     # [64, 512] per core
    cb_b[p, n]   = w_comp[:, b]^T @ onehot             # = w_comp[xtype[n], b]
    u_b[p, ihn]  = x^T * cb_b                          # scaled x halves
    outT[o, n]   = sum_{b,i} weight[b, i*256+o] * u
i.e. one dense K=4096 float32r matmul per core after a cheap on-device
type-lookup (broadcast-compare-matmul) and elementwise scale.

Sharding: data-parallel over N across 8 cores (512 rows each); weight and
w_comp replicated (w_comp uploaded column-replicated so each basis column
can be used as a stationary matmul operand). x is uploaded transposed and
the output comes back transposed - pure layout choices done at shard time
on host. All math (type lookup, scaling, matmuls) runs on device.

Matmuls run in float32r (TRN2's full-rate fp32 mode, ~1.2e-4 rounding).
"""

import sys

if "/opt/trn_rl_repo" not in sys.path:
    sys.path.insert(0, "/opt/trn_rl_repo")

import numpy as np

import concourse.bass as bass
import concourse.mybir as mybir
import concourse.tile as tile
from concourse import bacc
from concourse.bass_utils import run_bass_kernel_spmd

P = 128
N_FULL = 4096
IN_DIM = 256
OUT_DIM = 256
NUM_B = 16
NUM_T = 64
N_CORES = 8
ROWS = N_FULL // N_CORES          # 512 rows per core
KT = NUM_B * (IN_DIM // P)        # 32 contraction tiles of 128
GPS_BASES = frozenset({3, 7, 11, 14})  # bases whose scale-TT runs on gpsimd

F32 = mybir.dt.float32
F32R = mybir.dt.float32r
I32 = mybir.dt.int32


def _build_program():
    nc = bacc.Bacc("TRN2", target_bir_lowering=False, debug=False)

    xT = nc.declare_dram_parameter("xT", [P, 2 * ROWS], F32, isOutput=False)
    xtype = nc.declare_dram_parameter("xtype", [ROWS], I32, isOutput=False)
    iota_in = nc.declare_dram_parameter("iota_in", [NUM_T, 1], I32, isOutput=False)
    wcomp_bc = nc.declare_dram_parameter("wcomp_bc", [NUM_T, NUM_B * P], F32R, isOutput=False)
    weight = nc.declare_dram_parameter("weight", [NUM_B, IN_DIM * OUT_DIM], F32R, isOutput=False)
    outT = nc.declare_dram_parameter("outT", [OUT_DIM, ROWS], F32, isOutput=True)

    # weight chunk per b: [128, 2, 256], w_chunk[b][p, ih, o] = weight[b, (ih*128+p)*256 + o]
    wv = weight.ap().rearrange("b (ih p o) -> b p ih o", ih=2, p=P, o=OUT_DIM)

    with tile.TileContext(nc) as tc:
        with (
            tc.tile_pool(name="const", bufs=1) as constp,
            tc.tile_pool(name="wpool", bufs=1) as wpool,
            tc.tile_pool(name="cbp", bufs=2) as cbp,
            tc.tile_pool(name="up", bufs=5) as up,
            tc.tile_pool(name="outp", bufs=2) as outp,
            tc.tile_pool(name="psb", bufs=5, space="PSUM") as psb,
            tc.tile_pool(name="pso", bufs=1, space="PSUM") as pso,
        ):
            # ---- tiny inputs first: type ids (partition-broadcast), iota ----
            xtypeB = constp.tile([NUM_T, ROWS], I32, name="xtypeB")
            xtype_bcast = bass.AP(
                tensor=xtype.ap().tensor,
                offset=0,
                ap=[[0, NUM_T], [1, ROWS]],
            )
            nc.sync.dma_start(out=xtypeB[:], in_=xtype_bcast)
            iota_c = constp.tile([NUM_T, 1], I32, name="iota_c")
            nc.sync.dma_start(out=iota_c[:], in_=iota_in.ap()[:, :])

            wcb = constp.tile([NUM_T, NUM_B * P], F32R, name="wcb")
            nc.sync.dma_start(out=wcb[:], in_=wcomp_bc.ap()[:, :])

            xtcat = constp.tile([P, 2 * ROWS], F32, name="xtcat")
            nc.scalar.dma_start(out=xtcat, in_=xT.ap()[:, :])

            # weight chunks, resident; split across the two HWDGE queues
            wts = []
            for b in range(NUM_B):
                wt = wpool.tile([P, 2, OUT_DIM], F32R, name=f"w{b}")
                eng = nc.sync if b % 2 == 0 else nc.scalar
                eng.dma_start(out=wt, in_=wv[b])
                wts.append(wt)

            # ---- onehot[t, n] = (iota[t] == xtype[n]) ----
            onehot = constp.tile([NUM_T, ROWS], F32R, name="onehot")
            nc.vector.tensor_tensor(
                out=onehot[:],
                in0=iota_c[:].to_broadcast([NUM_T, ROWS]),
                in1=xtypeB[:],
                op=mybir.AluOpType.is_equal,
            )

            # ---- per-basis: cb = w_comp[:,b]-bcast ^T @ onehot; scale; matmul ----
            psums = [
                pso.tile([P, ROWS], F32, name=f"out{oh}", space="PSUM")
                for oh in range(2)
            ]

            def emit_cb(b):
                cb_ps = psb.tile([P, ROWS], F32, name="cbps", tag="cbps", space="PSUM")
                nc.tensor.matmul(
                    out=cb_ps[:],
                    lhsT=wcb[:, b * P : (b + 1) * P],
                    rhs=onehot[:],
                    start=True,
                    stop=True,
                )
                if b in GPS_BASES:
                    # gpsimd cannot read PSUM; stage via the scalar engine
                    cb_sb = cbp.tile([P, ROWS], F32, name="cbsb", tag="cbsb")
                    nc.scalar.copy(cb_sb[:], cb_ps[:])
                    return cb_sb
                return cb_ps

            cbs = {0: emit_cb(0), 1: emit_cb(1)}
            for b in range(NUM_B):
                if b + 2 < NUM_B:
                    cbs[b + 2] = emit_cb(b + 2)
                cb_src = cbs.pop(b)
                cb_rep = cb_src[:].rearrange("p (one n) -> p one n", one=1).to_broadcast(
                    [P, 2, ROWS]
                )
                u = up.tile([P, 2 * ROWS], F32R, name="u", tag="u")
                eng = nc.gpsimd if b in GPS_BASES else nc.vector
                eng.tensor_tensor(
                    out=u[:].rearrange("p (ih n) -> p ih n", ih=2),
                    in0=xtcat[:].rearrange("p (ih n) -> p ih n", ih=2),
                    in1=cb_rep,
                    op=mybir.AluOpType.mult,
                )
                for ih in range(2):
                    kt = b * 2 + ih
                    for oh in range(2):
                        nc.tensor.matmul(
                            out=psums[oh][:],
                            lhsT=wts[b][:, ih, oh * P : (oh + 1) * P],
                            rhs=u[:, ih * ROWS : (ih + 1) * ROWS],
                            start=(kt == 0),
                            stop=(kt == KT - 1),
                        )

            # ---- drain outT ----
            for oh in range(2):
                ot = outp.tile([P, ROWS], F32, name=f"ot{oh}")
                nc.scalar.copy(ot[:], psums[oh][:])
                eng = nc.sync if oh == 0 else nc.scalar
                eng.dma_start(out=outT.ap()[oh * P : (oh + 1) * P, :], in_=ot)

    nc.compile()
    return nc


_PROGRAM = None
LAST_RESULT = None  # test harness introspection


def kernel(x, xtype, weight, w_comp, trace=False):
    global _PROGRAM, LAST_RESULT
    x = np.asarray(x, dtype=np.float32)
    xtype = np.asarray(xtype)
    weight = np.asarray(weight, dtype=np.float32)
    w_comp = np.asarray(w_comp, dtype=np.float32)
    assert x.shape == (N_FULL, IN_DIM) and weight.shape == (NUM_B, IN_DIM * OUT_DIM)

    if _PROGRAM is None:
        _PROGRAM = _build_program()
    nc = _PROGRAM

    xtype32 = xtype.astype(np.int32)
    iota_c = np.arange(NUM_T, dtype=np.int32).reshape(NUM_T, 1)
    # w_comp columns replicated so each [64, 128] slice is a constant column
    wcomp_bc = np.ascontiguousarray(np.repeat(w_comp, P, axis=1))  # [64, 16*128]
    in_maps = []
    for c in range(N_CORES):
        s = slice(c * ROWS, (c + 1) * ROWS)
        in_maps.append(
            {
                "xT": np.ascontiguousarray(
                    x[s].T.reshape(2, P, ROWS).transpose(1, 0, 2).reshape(P, 2 * ROWS)
                ),
                "xtype": np.ascontiguousarray(xtype32[s]),
                "iota_in": iota_c,
                "wcomp_bc": wcomp_bc,
                "weight": weight,
            }
        )

    res = run_bass_kernel_spmd(nc, in_maps, list(range(N_CORES)), trace=trace)
    LAST_RESULT = res

    out = np.empty((N_FULL, OUT_DIM), np.float32)
    for c in range(N_CORES):
        s = slice(c * ROWS, (c + 1) * ROWS)
        out[s] = res.results[c]["outT"].T
    return out
